# revision 14
# baseline (speedup 1.0000x reference)
"""Angular prototypical loss on 8 TRN2 NeuronCores (Bass/Tile, SPMD).

kernel(**inputs): takes FULL inputs (embeddings [65536,256] f32, labels
[65536] i32, num_classes), shards the batch across the 8 cores, runs one
SPMD Bass kernel (AllReduce of per-class prototype sums on-chip), returns
the scalar mean loss.

v2 design (per core, 8192 rows = 64 tiles of 128):
- Phase A: load embeddings with an f32->bf16 casting DMA, row-normalize
  (TTR normsq -> recip -> sqrt -> scale), build one-hots on DVE (2x mode),
  accumulate protoT[d, c] in PSUM with 4 wide matmuls per tile
  (moving dim = 512 classes) instead of 8 narrow ones.
- AllReduce in 2 class-halves, pipelined with the normalize/transpose
  chain (square+reduce per class on the transposed C-major layout).
- Phase B: cos tiles via 4 matmuls into a 2-tile PSUM pair, one Exp
  activation per pair ([128, 2048]), sumexp via segmented DVE reduce,
  target-prototype rows fetched with one batched indirect gather per
  8-tile group, m = TTR dot per tile.
"""
import math

import numpy as np

import concourse.bass as bass
import concourse.bacc as bacc
import concourse.mybir as mybir
import concourse.tile as tile
from concourse.bass_utils import run_bass_kernel_spmd

P = 128
D = 256
C = 1024
NCORES = 8
MARGIN = 0.2
INV_T = 10.0
COS_M = math.cos(MARGIN)
SIN_M = math.sin(MARGIN)
TH = math.cos(math.pi - MARGIN)

f32 = mybir.dt.float32
bf16 = mybir.dt.bfloat16
fp16 = mybir.dt.float16
i32 = mybir.dt.int32

AF = mybir.ActivationFunctionType
OP = mybir.AluOpType


def build(nt: int = 64, group: int = 8, stage: int = 5):
    """stage: debug bisection — 1=Phase A, 2=+AR, 3=+normalize, 4=+Phase B
    (no gathers), 5=full."""
    BL = P * nt
    ng = nt // group
    assert nt % group == 0

    nc = bacc.Bacc("TRN2", target_bir_lowering=False, debug=False,
                   num_devices=NCORES)
    emb = nc.declare_dram_parameter("embeddings", [BL, D], f32, isOutput=False)
    lab = nc.declare_dram_parameter("labels", [BL], i32, isOutput=False)
    out = nc.declare_dram_parameter("out", [P, 1], f32, isOutput=True)

    emb_g = emb.ap().rearrange("(p q) d -> p q d", p=P)      # [128, nt, 256]
    lab_pn = lab.ap().rearrange("(p n) -> p n", p=P)         # [128, nt]

    with tile.TileContext(nc) as tc:
        with (
            tc.tile_pool(name="big", bufs=1) as big,
            tc.tile_pool(name="stage", bufs=2) as stg,
            tc.tile_pool(name="ohp", bufs=3) as ohp,
            tc.tile_pool(name="xpp", bufs=2) as xpp,
            tc.tile_pool(name="gat", bufs=2) as gat,
            tc.tile_pool(name="scr", bufs=2) as scr,
            tc.tile_pool(name="dram", bufs=1, space="DRAM") as dram,
        ):
            # ---- DRAM staging for the class-half AllReduces ----
            s_loc = [dram.tile([P, 2, C // 2], bf16, tag=f"s_loc{h}",
                               name=f"s_loc{h}") for h in range(2)]
            s_glob = [dram.tile([P, 2, C // 2], bf16, tag=f"s_glob{h}",
                                name=f"s_glob{h}", addr_space="Shared")
                      for h in range(2)]
            shat_dram = dram.tile([C, D], bf16, tag="shat_dram")

            # ---- persistent SBUF ----
            ehat = big.tile([P, nt, D], bf16, tag="ehat")
            eT = big.tile([P, nt, 2, P], bf16, tag="eT")
            sT = big.tile([P, 2, C], bf16, tag="sT")
            lab_i = big.tile([P, nt], i32, tag="lab_i")
            lab_f = big.tile([P, nt], f32, tag="lab_f")
            normsq = big.tile([P, nt], f32, tag="normsq")
            invn = big.tile([P, nt], f32, tag="invn")
            m_all = big.tile([P, nt], f32, tag="m_all")
            sumexp = big.tile([P, nt], f32, tag="sumexp")
            iota16 = big.tile([P, C], fp16, tag="iota16")

            nc.gpsimd.iota(iota16[:], pattern=[[1, C]], base=0,
                           channel_multiplier=0,
                           allow_small_or_imprecise_dtypes=True)
            nc.sync.dma_start(out=lab_i[:], in_=lab_pn)
            nc.vector.tensor_copy(lab_f[:], lab_i[:])

            # ================= Phase A =================
            with tc.tile_pool(name="psA", bufs=1, space="PSUM") as psA:
                # protoT accumulators: [d-chunk(2)] x [c-half(2)], each
                # [128, 512] f32 = 1 PSUM bank.
                proto_ps = [[psA.tile([P, C // 2], f32, tag=f"proto{ch}{h}",
                                      name=f"proto_ps{ch}{h}")
                             for h in range(2)] for ch in range(2)]
                for g in range(ng):
                    gsl = slice(g * group, (g + 1) * group)
                    ebf = stg.tile([P, group, D], bf16, tag="ebf")
                    # f32 -> bf16 casting DMA (gpsimd SWDGE can cast)
                    nc.gpsimd.dma_start(out=ebf[:], in_=emb_g[:, gsl, :])
                    for t in range(group):
                        n = g * group + t
                        sq = scr.tile([P, D], bf16, tag="sq")
                        nc.scalar.activation(
                            sq[:], ebf[:, t, :], AF.Square,
                            accum_out=normsq[:, n:n + 1])
                    tmp8 = scr.tile([P, group], f32, tag="tmp8")
                    nc.vector.reciprocal(tmp8[:], normsq[:, gsl])
                    nc.scalar.sqrt(invn[:, gsl], tmp8[:])
                    for t in range(group):
                        n = g * group + t
                        e_n = ehat[:, n, :]
                        nc.vector.tensor_scalar(
                            e_n, ebf[:, t, :], invn[:, n:n + 1], None, OP.mult)
                        oh = ohp.tile([P, C], bf16, tag="oh")
                        nc.vector.tensor_scalar(
                            oh[:], iota16[:], lab_f[:, n:n + 1], None,
                            OP.is_equal)
                        for ch in range(2):
                            for h in range(2):
                                nc.tensor.matmul(
                                    out=proto_ps[ch][h][:],
                                    lhsT=e_n[:, ch * P:(ch + 1) * P],
                                    rhs=oh[:, h * 512:(h + 1) * 512],
                                    start=(n == 0), stop=(n == nt - 1))
                    # one batched xbar transpose for the whole group
                    nc.sync.dma_start_transpose(
                        out=eT[:, gsl, :, :],
                        in_=ehat[:, gsl, :].rearrange("p g d -> p (g d)"))

                # ---- PSUM -> SBUF (bf16), per class-half ----
                s_sb = [big.tile([P, 2, C // 2], bf16, tag=f"s_sb{h}",
                                 name=f"s_sb{h}") for h in range(2)]
                for h in range(2):
                    for ch in range(2):
                        nc.vector.tensor_copy(s_sb[h][:, ch, :],
                                              proto_ps[ch][h][:])

            def _dbg_out(src_ap):
                dbg = big.tile([P, 1], f32, tag="dbg")
                nc.vector.tensor_copy(dbg[:], src_ap)
                nc.sync.dma_start(out=out[:], in_=dbg[:])

            if stage == 1:
                _dbg_out(s_sb[0][:, 0, 0:1])
            # ---- chunked AllReduce + normalize pipeline ----
            for h in range(2):
                nc.sync.dma_start(out=s_loc[h][:], in_=s_sb[h][:])
                nc.gpsimd.collective_compute(
                    "AllReduce", OP.add,
                    replica_groups=[list(range(NCORES))],
                    ins=[s_loc[h][:].opt()], outs=[s_glob[h][:].opt()])

            if stage == 2:
                s2d = xpp.tile([P, 2, C // 2], bf16, tag="s2d")
                nc.sync.dma_start(out=s2d[:], in_=s_glob[0][:])
                _dbg_out(s2d[:, 0, 0:1])
            for h in range(2):
                s2 = xpp.tile([P, 2, C // 2], bf16, tag="s2")
                nc.sync.dma_start(out=s2[:], in_=s_glob[h][:])
                # transpose to C-major: blocks (ch, cc) -> sC[c, ch, cc, p]
                sC = xpp.tile([P, 2, 4, P], bf16, tag="sC")
                nc.sync.dma_start_transpose(
                    out=sC[:].rearrange("c ch cc p -> c (ch cc) p"),
                    in_=s2[:].rearrange("p ch c -> p (ch c)"))
                sqC = xpp.tile([P, 2, 4, P], bf16, tag="sqC")
                nc.vector.tensor_tensor(sqC[:], sC[:], sC[:], op=OP.mult)
                pnsq = xpp.tile([P, 4], f32, tag="pnsq")
                nc.vector.reduce_sum(
                    pnsq[:], sqC[:].rearrange("c ch cc p -> c cc ch p"),
                    axis=mybir.AxisListType.XY)
                ptmp = xpp.tile([P, 4], f32, tag="ptmp")
                pinv = xpp.tile([P, 4], f32, tag="pinv")
                nc.vector.reciprocal(ptmp[:], pnsq[:])
                nc.scalar.sqrt(pinv[:], ptmp[:])
                shatC = xpp.tile([P, 2, 4, P], bf16, tag="shatC")
                for cc in range(4):
                    nc.vector.tensor_scalar(
                        shatC[:, :, cc, :], sC[:, :, cc, :],
                        pinv[:, cc:cc + 1], None, OP.mult)
                # store gather table rows (class-major, natural d order)
                for ch in range(2):
                    nc.sync.dma_start(
                        out=shat_dram[:].rearrange(
                            "(hh cc c) (ch p) -> hh ch c cc p",
                            hh=2, c=P, ch=2)[h, ch],
                        in_=shatC[:, ch, :, :])
                # transpose back to d-major for the Phase B rhs
                for ch in range(2):
                    nc.sync.dma_start_transpose(
                        out=sT[:, ch, h * 512:(h + 1) * 512].rearrange(
                            "p (cc c) -> p cc c", c=P),
                        in_=shatC[:, ch, :, :].rearrange("c cc p -> c (cc p)"))

            if stage == 3:
                _dbg_out(sT[:, 0, 0:1])
            # ================= Phase B =================
            npair = nt // 2
            with tc.tile_pool(name="psB", bufs=2, space="PSUM") as psB:
                for g in range(ng):
                    Gts = []
                    for t in range(group):
                        n = g * group + t
                        Gt = gat.tile([P, D], bf16, tag=f"G{t}",
                                      name=f"G_{n}")
                        if stage >= 5:
                            nc.gpsimd.indirect_dma_start(
                                out=Gt[:], out_offset=None,
                                in_=shat_dram[:],
                                in_offset=bass.IndirectOffsetOnAxis(
                                    ap=lab_i[:, n:n + 1], axis=0))
                        else:
                            nc.vector.memset(Gt[:], 0.01)
                        Gts.append(Gt)
                    for pr in range(group // 2):
                        pp = psB.tile([P, 2, C], f32, tag="pp")
                        for t in range(2):
                            n = g * group + pr * 2 + t
                            for ch in range(2):
                                for hh in range(2):
                                    nc.tensor.matmul(
                                        out=pp[:, t, hh * 512:(hh + 1) * 512],
                                        lhsT=eT[:, n, ch, :],
                                        rhs=sT[:, ch, hh * 512:(hh + 1) * 512],
                                        start=(ch == 0), stop=(ch == 1))
                        exps = scr.tile([P, 2, C], fp16, tag="exps")
                        nc.scalar.activation(
                            exps[:].rearrange("p t c -> p (t c)"),
                            pp[:].rearrange("p t c -> p (t c)"),
                            AF.Exp, scale=INV_T)
                        n0 = g * group + pr * 2
                        nc.vector.reduce_sum(
                            sumexp[:, n0:n0 + 2], exps[:],
                            axis=mybir.AxisListType.X)
                        for t in range(2):
                            n = n0 + t
                            mdf = scr.tile([P, D], bf16, tag="mdf")
                            nc.vector.tensor_tensor(
                                mdf[:], ehat[:, n, :], Gts[pr * 2 + t][:],
                                op=OP.mult)
                            nc.vector.reduce_sum(m_all[:, n:n + 1], mdf[:],
                                                 axis=mybir.AxisListType.X)

            # ================= epilogue (batched [P, nt]) ========
            b1 = big.tile([P, nt], f32, tag="b1")
            b2 = big.tile([P, nt], f32, tag="b2")
            b3 = big.tile([P, nt], f32, tag="b3")
            b4 = big.tile([P, nt], f32, tag="b4")
            mask = big.tile([P, nt], mybir.dt.uint8, tag="mask")
            phi_f = big.tile([P, nt], f32, tag="phi_f")

            nc.vector.tensor_tensor(b1[:], m_all[:], m_all[:], op=OP.mult)
            nc.vector.tensor_scalar(b1[:], b1[:], -1.0, 1.0, OP.mult, OP.add)
            nc.vector.tensor_scalar_max(b1[:], b1[:], 0.0)
            nc.scalar.sqrt(b2[:], b1[:])                        # sin
            nc.vector.tensor_scalar_mul(b3[:], m_all[:], COS_M)
            nc.vector.tensor_scalar(b2[:], b2[:], -SIN_M, None, OP.mult)
            nc.vector.tensor_add(b3[:], b3[:], b2[:])           # phi
            nc.vector.tensor_scalar(mask[:], m_all[:], TH, None, OP.is_gt)
            nc.vector.tensor_scalar(b4[:], m_all[:], -MARGIN, None, OP.add)
            nc.vector.select(phi_f[:], mask[:], b3[:], b4[:])
            nc.scalar.activation(b1[:], m_all[:], AF.Exp, scale=INV_T)
            nc.scalar.activation(b2[:], phi_f[:], AF.Exp, scale=INV_T)
            nc.vector.tensor_sub(b1[:], sumexp[:], b1[:])
            nc.vector.tensor_add(b1[:], b1[:], b2[:])           # Z
            nc.scalar.activation(b2[:], b1[:], AF.Ln, scale=1.0)
            nc.vector.tensor_scalar_mul(b3[:], phi_f[:], INV_T)
            nc.vector.tensor_sub(b2[:], b2[:], b3[:])           # nll
            part = big.tile([P, 1], f32, tag="part")
            nc.vector.reduce_sum(part[:], b2[:], axis=mybir.AxisListType.X)
            nc.sync.dma_start(out=out[:], in_=part[:])

    nc.compile()
    return nc


_NC_CACHE = {}


def kernel(embeddings, labels, num_classes=None, **_ignored):
    embeddings = np.ascontiguousarray(embeddings, dtype=np.float32)
    labels = np.ascontiguousarray(labels, dtype=np.int32)
    B = embeddings.shape[0]
    BL = B // NCORES

    if "nc" not in _NC_CACHE:
        _NC_CACHE["nc"] = build()
    nc = _NC_CACHE["nc"]

    in_maps = [{"embeddings": embeddings[i * BL:(i + 1) * BL],
                "labels": labels[i * BL:(i + 1) * BL]}
               for i in range(NCORES)]
    res = run_bass_kernel_spmd(nc, in_maps, list(range(NCORES)))
    total = 0.0
    for i in range(NCORES):
        total += res.results[i]["out"].astype(np.float64).sum()
    return np.float32(total / B)
